# revision 62
# baseline (speedup 1.0000x reference)
"""COIL-style retrieval scoring kernel for Trainium2 (8 NeuronCores, SPMD).

Problem: nn_BertForSemanticEmbedding_16973710754315
  out[q, n] = sum_{i>=1} mask[q,i] * max_j( where(qid[q,i]==did[n,j], qry[q,i]·doc[n,j], 0) )

Bucketed formulation:
  * Matches require qid == did, so they require qid % B == did % B.  Host
    groups tokens by bucket b = id % 21 (a pure per-token permutation +
    padding -- all matching still happens on device).  Each core then runs
    21 small block matmuls [120 padded q-tokens x 256 padded doc-slots]
    instead of the dense [2048 x 2048] product: ~6x fewer PSUM elements
    and ~5x fewer PE columns.
  * Exact-match extraction inside the matmul: 64-dim augmented tokens =
    [reps (32) | code C[id] (31, entries +-4) | shift].  The shift row
    (qry: -496, doc: 1.0) pre-subtracts BIG = 31*16 = 496, so a matching
    pair scores S exactly while the worst same-bucket non-match scores
    <= maxS + maxGram - 496 = 41 + 304 - 496 < -150 (seed-verified).
    Padded doc slots are all-zero -> they score exactly 0, reproducing the
    reference's where(...)-zeros, so the segmented max needs no relu pass
    (>=1 empty slot per cell is guaranteed by the pack-or-fallback check).
  * Drain: ONE VectorE segmented reduce_max per 4-bucket PSUM group,
    straight from PSUM into bf16 Rall[120, 21, 16].  (Measured DVE reduce
    throughput is ~1.04 ns/elem regardless of dtype or PSUM/SBUF source,
    so staging copies through ScalarE buy nothing.)  Groups are 2 PSUM
    banks each, triple buffered, so the PE never waits on drain latency.
  * Finale: 21 accumulating [120x16]x[120x16] matmuls with the
    bucket-scattered attention mask (host-built W2) as stationary
    -> psum [16 queries, 16 docs] -> VectorE copy -> DMA out, deferred two
    groups so they never stall the in-order PE queue.
  * Inputs stream as contiguous per-group chunks over THREE DMA queues
    (sync + scalar HW DGE, gpsimd SW DGE) so compute starts at the first
    chunk and the tail groups are never transfer-gated.
  * Inputs whose bucket histograms exceed the static pads fall back to the
    dense kernel below (never triggers for the reference distribution).
"""

import sys
import numpy as np

for _p in ("/opt/trn_rl_repo",):
    if _p not in sys.path:
        sys.path.insert(0, _p)

import ml_dtypes

BF16 = ml_dtypes.bfloat16

NQ, LQ = 16, 128
ND, LD = 128, 128
D = 32
VOCAB = 1000
NCORES = 8
DSHARD = ND // NCORES  # 16 docs per core
NQTOK = NQ * LQ
NDTOK = DSHARD * LD

# --- bucketed-path parameters ---
B = 21                 # buckets: id % 21
QPAD = 120             # query-token slots per bucket (actual max 113)
DPAD = 16              # doc-token slots per (doc, bucket); >=1 kept empty
R = 31                 # code dims
KAUG = 64              # 32 reps + 31 code + 1 shift row
BIG = float(R * 16)    # 496 = exact code self-dot
SEED = 41              # verified: max same-bucket |Gram offdiag| = 304
DCOLS = DSHARD * DPAD  # 256 psum cols per bucket block
SLOT = 256             # f32 per block slot (1 KiB, bank-aligned pairs)
GROUPS = [1, 4, 4, 4, 4, 2, 2]
GMAX = max(GROUPS)
QCHUNKS = [[0], [1], [2], [3, 4, 5, 6]]
DCHUNKS = [[0], [1], [2], [3], [4], [5, 6]]
WARMUP = 8
FDEFER = 2             # finale for group g emitted during group g+FDEFER

_CODE = None


def _code():
    global _CODE
    if _CODE is None:
        rng = np.random.RandomState(SEED)
        _CODE = np.where(rng.rand(VOCAB, R) < 0.5, -4.0, 4.0).astype(np.float32)
    return _CODE


def _build_bucketed():
    from concourse import bacc, tile, mybir

    bf = mybir.dt.bfloat16
    f32 = mybir.dt.float32
    AX = mybir.AxisListType.X

    nc = bacc.Bacc("TRN2", target_bir_lowering=False, debug=False,
                   num_devices=NCORES)
    warm = nc.alloc_sbuf_tensor("warm", [KAUG, 2], f32)
    nc.gpsimd.memset(warm.ap(), 0.0)

    # one contiguous dram param per DMA chunk: strided slices of one big
    # tensor DMA at ~1/3 the bandwidth of contiguous blocks
    qT_ds = [nc.declare_dram_parameter(
        f"qT{c}", [KAUG, sum(GROUPS[g] for g in gs) * QPAD], bf,
        isOutput=False) for c, gs in enumerate(QCHUNKS)]
    dT_ds = [nc.declare_dram_parameter(
        f"dT{c}", [KAUG, sum(GROUPS[g] for g in gs) * DCOLS], bf,
        isOutput=False) for c, gs in enumerate(DCHUNKS)]
    w2_d = nc.declare_dram_parameter("w2", [QPAD, B * NQ], bf, isOutput=False)
    out_d = nc.declare_dram_parameter("out", [NQ, DSHARD], f32, isOutput=True)

    with tile.TileContext(nc) as tc:
        with (
            tc.tile_pool(name="io", bufs=1) as io,
            tc.tile_pool(name="ps", bufs=3, space="PSUM") as ps,
            tc.tile_pool(name="pso", bufs=1, space="PSUM") as pso,
        ):
            psO = pso.tile([NQ, DSHARD], f32, tag="pso")
            wap = warm.ap()
            # PE p-state warm-up on preamble-initialized scratch
            for _ in range(WARMUP):
                nc.tensor.matmul(psO[0:1, 0:1], wap[:, 0:1], wap[:, 0:1],
                                 start=True, stop=True)

            # per-chunk input tiles; chunked DMA so group 0 lands first
            QTc = [io.tile([KAUG, sum(GROUPS[g] for g in gs) * QPAD], bf,
                           tag=f"qt{c}", name=f"QTt{c}")
                   for c, gs in enumerate(QCHUNKS)]
            DTc = [io.tile([KAUG, sum(GROUPS[g] for g in gs) * DCOLS], bf,
                           tag=f"dt{c}", name=f"DTt{c}")
                   for c, gs in enumerate(DCHUNKS)]
            # group -> (chunk tile, column offset within chunk)
            qmap, dmap = {}, {}
            for c, gs in enumerate(QCHUNKS):
                off = 0
                for g in gs:
                    qmap[g] = (QTc[c], off)
                    off += GROUPS[g] * QPAD
            for c, gs in enumerate(DCHUNKS):
                off = 0
                for g in gs:
                    dmap[g] = (DTc[c], off)
                    off += GROUPS[g] * DCOLS
            W2 = io.tile([QPAD, B, NQ], bf, tag="w2")
            Rall = io.tile([QPAD, B, DSHARD], bf, tag="rall")
            OUTS = io.tile([NQ, DSHARD], f32, tag="outs")

            # dma priority: dT chunks stream on the scalar queue; qT (small
            # group-0 chunk, then the fat rest), W2, and the last dT chunk
            # ride sync.
            nc.scalar.dma_start(DTc[0][:], dT_ds[0][:])
            nc.sync.dma_start(QTc[0][:], qT_ds[0][:])
            nc.gpsimd.dma_start(QTc[2][:], qT_ds[2][:])
            nc.scalar.dma_start(DTc[1][:], dT_ds[1][:])
            nc.sync.dma_start(QTc[1][:], qT_ds[1][:])
            nc.gpsimd.dma_start(DTc[2][:], dT_ds[2][:])
            nc.scalar.dma_start(DTc[3][:], dT_ds[3][:])
            nc.sync.dma_start(QTc[3][:], qT_ds[3][:])
            nc.gpsimd.dma_start(DTc[4][:], dT_ds[4][:])
            nc.scalar.dma_start(DTc[5][:], dT_ds[5][:])
            nc.sync.dma_start(W2[:].rearrange("p b q -> p (b q)"), w2_d[:])

            def emit_finale(b0, L):
                for k in range(L):
                    b = b0 + k
                    nc.tensor.matmul(psO[:], W2[:, b, :], Rall[:, b, :],
                                     start=(b == 0), stop=(b == B - 1),
                                     skip_group_check=True)

            fin_pending = []
            b0 = 0
            for g, L in enumerate(GROUPS):
                Pg = ps.tile([QPAD, GMAX, SLOT], f32, tag="ps")
                qt, qoff = qmap[g]
                dt, doff = dmap[g]
                for k in range(L):
                    nc.tensor.matmul(
                        Pg[:, k, 0:DCOLS],
                        qt[:, qoff + k * QPAD:qoff + (k + 1) * QPAD],
                        dt[:, doff + k * DCOLS:doff + (k + 1) * DCOLS],
                        start=True, stop=True)
                if len(fin_pending) >= FDEFER:
                    emit_finale(*fin_pending.pop(0))
                # the whole drain: one segmented reduce_max straight from
                # PSUM (measured DVE reduce rate is ~1.04 ns/elem from PSUM
                # and from bf16 SBUF alike, so staging through ScalarE
                # copies buys nothing)
                view = Pg[:, 0:L, 0:DCOLS].rearrange(
                    "p l (n j) -> p l n j", n=DSHARD)
                nc.vector.reduce_max(Rall[:, b0:b0 + L, :], view, axis=AX)
                fin_pending.append((b0, L))
                b0 += L

            for args in fin_pending:
                emit_finale(*args)
            # OUTS copy on VectorE: keeps ScalarE entirely out of the
            # program (no ACT table load in the preamble)
            nc.vector.tensor_scalar_max(OUTS[:], psO[:], -3.0e38)
            nc.sync.dma_start(out_d[:], OUTS[:])

    nc.compile()
    return nc


def _pack_bucketed(doc_reps, qry_reps, qry_attention_mask, doc_input_ids,
                   qry_input_ids):
    """Bucket + pad all tokens. Returns per-core in_maps, or None if any
    bucket cell exceeds the static pads (-> dense fallback)."""
    C = _code()
    qreps = np.asarray(qry_reps, np.float32).reshape(NQTOK, D)
    dreps = np.asarray(doc_reps, np.float32).reshape(ND * LD, D)
    qid = np.asarray(qry_input_ids).astype(np.int64).reshape(-1)
    did = np.asarray(doc_input_ids).astype(np.int64)
    mask = np.asarray(qry_attention_mask, np.float32)

    qb = qid % B
    idx = np.argsort(qb, kind="stable")
    sb = qb[idx]
    pos = np.arange(NQTOK) - np.searchsorted(sb, sb)
    if len(pos) and pos.max() >= QPAD:
        return None
    Qaug = np.zeros((B, QPAD, KAUG), np.float32)
    Qaug[sb, pos, 0:D] = qreps[idx]
    Qaug[sb, pos, D:D + R] = C[qid[idx]]
    Qaug[sb, pos, KAUG - 1] = -BIG
    qT = np.ascontiguousarray(
        Qaug.reshape(B * QPAD, KAUG).T).astype(BF16)
    gq = np.cumsum([0] + [L * QPAD for L in GROUPS])
    qT_chunks = {}
    for c, gs in enumerate(QCHUNKS):
        qT_chunks[f"qT{c}"] = np.ascontiguousarray(
            np.concatenate([qT[:, gq[g]:gq[g + 1]] for g in gs], axis=1))

    W = mask.copy()
    W[:, 0] = 0.0  # skip [CLS]
    W2big = np.zeros((QPAD, B, NQ), np.float32)
    W2big[pos, sb, idx // LQ] = W.reshape(-1)[idx]
    w2 = np.ascontiguousarray(W2big.reshape(QPAD, B * NQ)).astype(BF16)

    in_maps = []
    for core in range(NCORES):
        dd = did[core * DSHARD:(core + 1) * DSHARD]
        dr = dreps[core * NDTOK:(core + 1) * NDTOK]
        db = (dd.reshape(-1) % B)
        docn = np.repeat(np.arange(DSHARD), LD)
        key = db * DSHARD + docn
        didx = np.argsort(key, kind="stable")
        sk = key[didx]
        dpos = np.arange(NDTOK) - np.searchsorted(sk, sk)
        # keep >= 1 all-zero slot per cell: it reproduces the reference's
        # where()-zeros so no relu pass is needed on device
        if dpos.max() > DPAD - 2:
            return None
        Daug = np.zeros((B, DSHARD, DPAD, KAUG), np.float32)
        sb2 = sk // DSHARD
        sn = sk % DSHARD
        ids_flat = dd.reshape(-1)[didx]
        Daug[sb2, sn, dpos, 0:D] = dr[didx]
        Daug[sb2, sn, dpos, D:D + R] = C[ids_flat]
        Daug[sb2, sn, dpos, KAUG - 1] = 1.0
        dT = np.ascontiguousarray(
            Daug.reshape(B * DCOLS, KAUG).T).astype(BF16)
        im = {"w2": w2}
        im.update(qT_chunks)
        gd = np.cumsum([0] + [L * DCOLS for L in GROUPS])
        for c, gs in enumerate(DCHUNKS):
            im[f"dT{c}"] = np.ascontiguousarray(
                np.concatenate([dT[:, gd[g]:gd[g + 1]] for g in gs], axis=1))
        in_maps.append(im)
    return in_maps


# ---------------------------------------------------------------------------
# Dense fallback (previous kernel): exact for any input distribution.
# ---------------------------------------------------------------------------

DR = 96                  # dense-path signature code dims
DCVAL = 4.0
DBIG = DR * DCVAL * DCVAL
DKAUG = D + DR           # 128
PA_DOCS = 4
F_DVE = 4
F_ACT = DSHARD - F_DVE
QBATCH = 4

_DENSE_CODE = None


def _dense_code():
    global _DENSE_CODE
    if _DENSE_CODE is None:
        rng = np.random.RandomState(12345)
        _DENSE_CODE = np.where(rng.rand(VOCAB, DR) < 0.5, -DCVAL,
                               DCVAL).astype(np.float32)
    return _DENSE_CODE


def _build_dense():
    from concourse import bacc, tile, mybir

    bf = mybir.dt.bfloat16
    f32 = mybir.dt.float32

    nc = bacc.Bacc("TRN2", target_bir_lowering=False, debug=False,
                   num_devices=NCORES)
    _bias_t = nc.alloc_sbuf_tensor("const-float32--1536", [128, 1],
                                   mybir.dt.float32)
    nc.gpsimd.memset(_bias_t.ap(), -float(DBIG))
    nc.const_aps.aps[(mybir.dt.float32, -float(DBIG))] = _bias_t.ap()

    qT_d = nc.declare_dram_parameter("qT", [DKAUG, NQTOK], bf, isOutput=False)
    dT_d = nc.declare_dram_parameter("dT", [DKAUG, NDTOK], bf, isOutput=False)
    w2_d = nc.declare_dram_parameter("w2", [LQ, NQ], bf, isOutput=False)
    out_d = nc.declare_dram_parameter("out", [1, NQ * DSHARD], f32,
                                      isOutput=True)

    NCHUNK = 4
    CW = NDTOK // NCHUNK

    with tile.TileContext(nc) as tc:
        with (
            tc.tile_pool(name="io", bufs=1) as io,
            tc.tile_pool(name="ebuf", bufs=2) as ebuf,
            tc.tile_pool(name="small", bufs=1) as small,
            tc.tile_pool(name="psA", bufs=2, space="PSUM") as psA,
            tc.tile_pool(name="psB", bufs=2, space="PSUM") as psB,
        ):
            scr = psB.tile([1, 16], f32, tag="psB")
            bias_ap = _bias_t.ap()
            for _ in range(48):
                nc.tensor.matmul(scr[:, 0:1], bias_ap[:, 0:1], bias_ap[:, 0:1],
                                 start=True, stop=True)

            qchunk = NQTOK // NCHUNK
            DTt = io.tile([DKAUG, NDTOK], bf, tag="dt")
            nc.sync.dma_start(DTt[:], dT_d[:])
            DTc = [DTt[:, c * CW:(c + 1) * CW] for c in range(NCHUNK)]
            QT0 = io.tile([DKAUG, qchunk], bf, tag="qt0")
            nc.scalar.dma_start(QT0[:], qT_d[:, 0:qchunk])
            QTrest = io.tile([DKAUG, NQTOK - qchunk], bf, tag="qtr")
            nc.scalar.dma_start(QTrest[:], qT_d[:, qchunk:NQTOK])
            W2 = small.tile([LQ, NQ], bf, tag="w2")
            nc.scalar.dma_start(W2[:], w2_d[:])

            Mdve = small.tile([LQ, NQ, F_DVE], f32, tag="mdve")
            Rall = small.tile([LQ, NQ, DSHARD], bf, tag="rall")
            OUTS = small.tile([1, NQ * DSHARD], f32, tag="outs")

            docs_per_chunk = CW // LD
            q_per_chunk = qchunk // LQ

            BS = [5, 5, 4, 2]
            fin_deferred = []
            deep_deferred = []

            def emit_finale(qlo, qhi):
                n = (qhi - qlo) * DSHARD
                pso_b = psA.tile([1, n], f32, tag="psA")
                for q in range(qlo, qhi):
                    nc.tensor.matmul(
                        pso_b[:, (q - qlo) * DSHARD:(q - qlo + 1) * DSHARD],
                        W2[:, q:q + 1], Rall[:, q, :],
                        start=True, stop=True)
                nc.scalar.copy(OUTS[:, qlo * DSHARD:qhi * DSHARD], pso_b[:])

            def emit_deep(T1, qlo, qhi):
                bs = qhi - qlo
                cur = T1
                width = LD // 2
                lev = 1
                while width > 2:
                    half = width // 2
                    t = ebuf.tile([LQ, bs, F_ACT, half], bf, tag=f"tr{lev}")
                    nc.vector.tensor_max(t[:], cur[:, :, :, 0:half],
                                         cur[:, :, :, half:width])
                    cur = t[:]
                    width = half
                    lev += 1
                nc.vector.tensor_max(Rall[:, qlo:qhi, F_DVE:DSHARD],
                                     cur[:, :, :, 0], cur[:, :, :, 1])

            qbase = 0
            for bi, bs in enumerate(BS):
                if fin_deferred and len(fin_deferred) > 1:
                    emit_finale(*fin_deferred.pop(0))
                E4 = ebuf.tile([LQ, bs, F_ACT, LD], bf, tag="e")
                T1 = ebuf.tile([LQ, bs, F_ACT, LD // 2], bf, tag="t1")
                for qq in range(bs):
                    q = qbase + qq
                    pA = psA.tile([LQ, PA_DOCS, LD], f32, tag="psA")
                    pB = psB.tile([LQ, DSHARD - PA_DOCS, LD], f32, tag="psB")
                    if q < q_per_chunk:
                        lhs = QT0[:, q * LQ:(q + 1) * LQ]
                    else:
                        lhs = QTrest[:, (q - q_per_chunk) * LQ:
                                     (q - q_per_chunk + 1) * LQ]
                    nc.tensor.matmul(pA[:], lhs, DTc[0],
                                     start=True, stop=True)
                    for c in range(1, NCHUNK):
                        nc.tensor.matmul(
                            pB[:, (c - 1) * docs_per_chunk:c * docs_per_chunk, :],
                            lhs, DTc[c], start=True, stop=True)
                    from concourse import mybir as _mb
                    nc.vector.reduce_max(
                        Mdve[:, q, :], pA[:, 0:F_DVE, :],
                        axis=_mb.AxisListType.X)
                    nc.scalar.activation(
                        E4[:, qq, :, :], pB[:],
                        _mb.ActivationFunctionType.Relu,
                        bias=-float(DBIG))
                    nc.vector.tensor_max(
                        T1[:, qq, :, :],
                        E4[:, qq, :, 0:LD // 2], E4[:, qq, :, LD // 2:LD])
                    if qq == 1 and deep_deferred:
                        emit_deep(*deep_deferred.pop(0))

                qlo, qhi = qbase, qbase + bs
                from concourse import mybir as _mb
                nc.scalar.activation(
                    Rall[:, qlo:qhi, 0:F_DVE], Mdve[:, qlo:qhi, :],
                    _mb.ActivationFunctionType.Relu,
                    bias=-float(DBIG))
                deep_deferred.append((T1[:], qlo, qhi))
                fin_deferred.append((qlo, qhi))
                qbase = qhi

            for args in deep_deferred:
                emit_deep(*args)
            for args in fin_deferred:
                emit_finale(*args)
            nc.sync.dma_start(out_d[:], OUTS[:])

    nc.compile()
    return nc


def _pack_dense(doc_reps, qry_reps, qry_attention_mask, doc_input_ids,
                qry_input_ids):
    C = _dense_code()
    qry_reps = np.asarray(qry_reps, dtype=np.float32)
    doc_reps = np.asarray(doc_reps, dtype=np.float32)
    mask = np.asarray(qry_attention_mask, dtype=np.float32)
    qids = np.asarray(qry_input_ids).astype(np.int64).reshape(-1)
    dids = np.asarray(doc_input_ids).astype(np.int64).reshape(-1)

    Qaug = np.concatenate(
        [qry_reps.reshape(NQTOK, D), C[qids]], axis=1).astype(BF16)
    Daug = np.concatenate(
        [doc_reps.reshape(ND * LD, D), C[dids]], axis=1).astype(BF16)
    qT = np.ascontiguousarray(Qaug.T)

    W = mask.copy()
    W[:, 0] = 0.0
    w2 = np.ascontiguousarray(W.T).astype(BF16)

    in_maps = []
    for core in range(NCORES):
        shard = Daug[core * NDTOK:(core + 1) * NDTOK]
        dT = np.ascontiguousarray(shard.T)
        in_maps.append({"qT": qT, "dT": dT, "w2": w2})
    return in_maps


# ---------------------------------------------------------------------------

_NC_BUCKET = None
_NC_DENSE = None


def _get_nc_bucketed():
    global _NC_BUCKET
    if _NC_BUCKET is None:
        _NC_BUCKET = _build_bucketed()
    return _NC_BUCKET


def _get_nc_dense():
    global _NC_DENSE
    if _NC_DENSE is None:
        _NC_DENSE = _build_dense()
    return _NC_DENSE


def _install_ntff_shim():
    """Under axon the NTFF profile hook module may be missing; install it so
    trace=True returns exec_time_ns. Harmless no-op if already present."""
    import types
    try:
        import antenv.axon_hooks  # noqa: F401
        return
    except ImportError:
        pass
    try:
        from trn_agent_boot.trn_boot import _ntff_profile_via_ctypes
        hook = _ntff_profile_via_ctypes("/opt/axon/libaxon_pjrt.so")
        mod = types.ModuleType("antenv.axon_hooks")
        mod.get_axon_ntff_profile_hook = lambda: hook
        mod.set_axon_ntff_profile_hook = lambda h: None
        sys.modules["antenv.axon_hooks"] = mod
    except Exception:
        pass


def _run(nc, in_maps, trace=False):
    from concourse.bass_utils import run_bass_kernel_spmd
    if trace:
        _install_ntff_shim()
    return run_bass_kernel_spmd(nc, in_maps, core_ids=list(range(NCORES)),
                                trace=trace)


def _kernel_impl(inputs, trace):
    in_maps = _pack_bucketed(**inputs)
    if in_maps is not None:
        res = _run(_get_nc_bucketed(), in_maps, trace=trace)
        out = np.zeros((NQ, ND), dtype=np.float32)
        for core in range(NCORES):
            out[:, core * DSHARD:(core + 1) * DSHARD] = res.results[core]["out"]
        return out, res
    in_maps = _pack_dense(**inputs)
    res = _run(_get_nc_dense(), in_maps, trace=trace)
    out = np.zeros((NQ, ND), dtype=np.float32)
    for core in range(NCORES):
        out[:, core * DSHARD:(core + 1) * DSHARD] = \
            res.results[core]["out"].reshape(NQ, DSHARD)
    return out, res


def kernel(doc_reps, qry_reps, qry_attention_mask, doc_input_ids,
           qry_input_ids):
    out, _ = _kernel_impl(dict(
        doc_reps=doc_reps, qry_reps=qry_reps,
        qry_attention_mask=qry_attention_mask,
        doc_input_ids=doc_input_ids, qry_input_ids=qry_input_ids), False)
    return out


def kernel_traced(doc_reps, qry_reps, qry_attention_mask, doc_input_ids,
                  qry_input_ids):
    """Returns (output, exec_time_ns) using the NTFF profiling path."""
    out, res = _kernel_impl(dict(
        doc_reps=doc_reps, qry_reps=qry_reps,
        qry_attention_mask=qry_attention_mask,
        doc_input_ids=doc_input_ids, qry_input_ids=qry_input_ids), True)
    return out, res.exec_time_ns


# revision 64
# speedup vs baseline: 1.0304x; 1.0304x over previous
"""COIL-style retrieval scoring kernel for Trainium2 (8 NeuronCores, SPMD).

Problem: nn_BertForSemanticEmbedding_16973710754315
  out[q, n] = sum_{i>=1} mask[q,i] * max_j( where(qid[q,i]==did[n,j], qry[q,i]·doc[n,j], 0) )

Bucketed formulation:
  * Matches require qid == did, so they require qid % B == did % B.  Host
    groups tokens by bucket b = id % 21 (a pure per-token permutation +
    padding -- all matching still happens on device).  Each core then runs
    21 small block matmuls [120 padded q-tokens x 256 padded doc-slots]
    instead of the dense [2048 x 2048] product: ~6x fewer PSUM elements
    and ~5x fewer PE columns.
  * Exact-match extraction inside the matmul: 64-dim augmented tokens =
    [reps (32) | code C[id] (31, entries +-4) | shift].  The shift row
    (qry: -496, doc: 1.0) pre-subtracts BIG = 31*16 = 496, so a matching
    pair scores S exactly while the worst same-bucket non-match scores
    <= maxS + maxGram - 496 = 41 + 304 - 496 < -150 (seed-verified).
    Padded doc slots are all-zero -> they score exactly 0, reproducing the
    reference's where(...)-zeros, so the segmented max needs no relu pass
    (>=1 empty slot per cell is guaranteed by the pack-or-fallback check).
  * Drain: ONE VectorE segmented reduce_max per 4-bucket PSUM group,
    straight from PSUM into bf16 Rall[120, 21, 16].  (Measured DVE reduce
    throughput is ~1.04 ns/elem regardless of dtype or PSUM/SBUF source,
    so staging copies through ScalarE buy nothing.)  Groups are 2 PSUM
    banks each, triple buffered, so the PE never waits on drain latency.
  * Finale: 21 accumulating [120x16]x[120x16] matmuls with the
    bucket-scattered attention mask (host-built W2) as stationary
    -> psum [16 queries, 16 docs] -> VectorE copy -> DMA out, deferred two
    groups so they never stall the in-order PE queue.
  * Inputs stream as contiguous per-group chunks over THREE DMA queues
    (sync + scalar HW DGE, gpsimd SW DGE) so compute starts at the first
    chunk and the tail groups are never transfer-gated.
  * Inputs whose bucket histograms exceed the static pads fall back to the
    dense kernel below (never triggers for the reference distribution).
"""

import sys
import numpy as np

for _p in ("/opt/trn_rl_repo",):
    if _p not in sys.path:
        sys.path.insert(0, _p)

import ml_dtypes

BF16 = ml_dtypes.bfloat16

NQ, LQ = 16, 128
ND, LD = 128, 128
D = 32
VOCAB = 1000
NCORES = 8
DSHARD = ND // NCORES  # 16 docs per core
NQTOK = NQ * LQ
NDTOK = DSHARD * LD

# --- bucketed-path parameters ---
B = 21                 # buckets: id % 21
QPAD = 120             # query-token slots per bucket (actual max 113)
DPAD = 16              # doc-token slots per (doc, bucket); >=1 kept empty
R = 31                 # code dims
KAUG = 64              # 32 reps + 31 code + 1 shift row
BIG = float(R * 16)    # 496 = exact code self-dot
SEED = 41              # verified: max same-bucket |Gram offdiag| = 304
DCOLS = DSHARD * DPAD  # 256 psum cols per bucket block
SLOT = 256             # f32 per block slot (1 KiB, bank-aligned pairs)
GROUPS = [1, 4, 4, 4, 4, 2, 2]
GMAX = max(GROUPS)
QCHUNKS = [[0], [1, 2], [3, 4, 5, 6]]
DCHUNKS = [[0], [1], [2], [3], [4], [5, 6]]
WARMUP = 8
FDEFER = 2             # finale for group g emitted during group g+FDEFER

_CODE = None


def _code():
    global _CODE
    if _CODE is None:
        rng = np.random.RandomState(SEED)
        _CODE = np.where(rng.rand(VOCAB, R) < 0.5, -4.0, 4.0).astype(np.float32)
    return _CODE


def _build_bucketed():
    from concourse import bacc, tile, mybir

    bf = mybir.dt.bfloat16
    f32 = mybir.dt.float32
    AX = mybir.AxisListType.X

    nc = bacc.Bacc("TRN2", target_bir_lowering=False, debug=False,
                   num_devices=NCORES)
    warm = nc.alloc_sbuf_tensor("warm", [KAUG, 2], f32)
    nc.gpsimd.memset(warm.ap(), 0.0)

    # one contiguous dram param per DMA chunk: strided slices of one big
    # tensor DMA at ~1/3 the bandwidth of contiguous blocks
    qT_ds = [nc.declare_dram_parameter(
        f"qT{c}", [KAUG, sum(GROUPS[g] for g in gs) * QPAD], bf,
        isOutput=False) for c, gs in enumerate(QCHUNKS)]
    dT_ds = [nc.declare_dram_parameter(
        f"dT{c}", [KAUG, sum(GROUPS[g] for g in gs) * DCOLS], bf,
        isOutput=False) for c, gs in enumerate(DCHUNKS)]
    w2_d = nc.declare_dram_parameter("w2", [QPAD, B * NQ], bf, isOutput=False)
    out_d = nc.declare_dram_parameter("out", [NQ, DSHARD], f32, isOutput=True)

    with tile.TileContext(nc) as tc:
        with (
            tc.tile_pool(name="io", bufs=1) as io,
            tc.tile_pool(name="ps", bufs=3, space="PSUM") as ps,
            tc.tile_pool(name="pso", bufs=1, space="PSUM") as pso,
        ):
            psO = pso.tile([NQ, DSHARD], f32, tag="pso")
            wap = warm.ap()
            # PE p-state warm-up on preamble-initialized scratch
            for _ in range(WARMUP):
                nc.tensor.matmul(psO[0:1, 0:1], wap[:, 0:1], wap[:, 0:1],
                                 start=True, stop=True)

            # per-chunk input tiles; chunked DMA so group 0 lands first
            QTc = [io.tile([KAUG, sum(GROUPS[g] for g in gs) * QPAD], bf,
                           tag=f"qt{c}", name=f"QTt{c}")
                   for c, gs in enumerate(QCHUNKS)]
            DTc = [io.tile([KAUG, sum(GROUPS[g] for g in gs) * DCOLS], bf,
                           tag=f"dt{c}", name=f"DTt{c}")
                   for c, gs in enumerate(DCHUNKS)]
            # group -> (chunk tile, column offset within chunk)
            qmap, dmap = {}, {}
            for c, gs in enumerate(QCHUNKS):
                off = 0
                for g in gs:
                    qmap[g] = (QTc[c], off)
                    off += GROUPS[g] * QPAD
            for c, gs in enumerate(DCHUNKS):
                off = 0
                for g in gs:
                    dmap[g] = (DTc[c], off)
                    off += GROUPS[g] * DCOLS
            W2 = io.tile([QPAD, B, NQ], bf, tag="w2")
            Rall = io.tile([QPAD, B, DSHARD], bf, tag="rall")
            OUTS = io.tile([NQ, DSHARD], f32, tag="outs")

            # dma priority: dT chunks stream on the scalar queue; qT (small
            # group-0 chunk, then the fat rest), W2, and the last dT chunk
            # ride sync.
            nc.scalar.dma_start(DTc[0][:], dT_ds[0][:])
            nc.sync.dma_start(QTc[0][:], qT_ds[0][:])
            nc.gpsimd.dma_start(DTc[2][:], dT_ds[2][:])
            nc.scalar.dma_start(DTc[1][:], dT_ds[1][:])
            nc.sync.dma_start(QTc[1][:], qT_ds[1][:])
            nc.gpsimd.dma_start(DTc[4][:], dT_ds[4][:])
            nc.scalar.dma_start(DTc[3][:], dT_ds[3][:])
            nc.sync.dma_start(QTc[2][:], qT_ds[2][:])
            nc.scalar.dma_start(DTc[5][:], dT_ds[5][:])
            nc.sync.dma_start(W2[:].rearrange("p b q -> p (b q)"), w2_d[:])

            def emit_finale(b0, L):
                for k in range(L):
                    b = b0 + k
                    nc.tensor.matmul(psO[:], W2[:, b, :], Rall[:, b, :],
                                     start=(b == 0), stop=(b == B - 1),
                                     skip_group_check=True)

            fin_pending = []
            b0 = 0
            for g, L in enumerate(GROUPS):
                Pg = ps.tile([QPAD, GMAX, SLOT], f32, tag="ps")
                qt, qoff = qmap[g]
                dt, doff = dmap[g]
                for k in range(L):
                    nc.tensor.matmul(
                        Pg[:, k, 0:DCOLS],
                        qt[:, qoff + k * QPAD:qoff + (k + 1) * QPAD],
                        dt[:, doff + k * DCOLS:doff + (k + 1) * DCOLS],
                        start=True, stop=True)
                if len(fin_pending) >= FDEFER:
                    emit_finale(*fin_pending.pop(0))
                # the whole drain: one segmented reduce_max straight from
                # PSUM (measured DVE reduce rate is ~1.04 ns/elem from PSUM
                # and from bf16 SBUF alike, so staging through ScalarE
                # copies buys nothing)
                view = Pg[:, 0:L, 0:DCOLS].rearrange(
                    "p l (n j) -> p l n j", n=DSHARD)
                nc.vector.reduce_max(Rall[:, b0:b0 + L, :], view, axis=AX)
                fin_pending.append((b0, L))
                b0 += L

            for args in fin_pending:
                emit_finale(*args)
            # OUTS copy on VectorE: keeps ScalarE entirely out of the
            # program (no ACT table load in the preamble)
            nc.vector.tensor_scalar_max(OUTS[:], psO[:], -3.0e38)
            nc.sync.dma_start(out_d[:], OUTS[:])

    nc.compile()
    return nc


def _pack_bucketed(doc_reps, qry_reps, qry_attention_mask, doc_input_ids,
                   qry_input_ids):
    """Bucket + pad all tokens. Returns per-core in_maps, or None if any
    bucket cell exceeds the static pads (-> dense fallback)."""
    C = _code()
    qreps = np.asarray(qry_reps, np.float32).reshape(NQTOK, D)
    dreps = np.asarray(doc_reps, np.float32).reshape(ND * LD, D)
    qid = np.asarray(qry_input_ids).astype(np.int64).reshape(-1)
    did = np.asarray(doc_input_ids).astype(np.int64)
    mask = np.asarray(qry_attention_mask, np.float32)

    qb = qid % B
    idx = np.argsort(qb, kind="stable")
    sb = qb[idx]
    pos = np.arange(NQTOK) - np.searchsorted(sb, sb)
    if len(pos) and pos.max() >= QPAD:
        return None
    Qaug = np.zeros((B, QPAD, KAUG), np.float32)
    Qaug[sb, pos, 0:D] = qreps[idx]
    Qaug[sb, pos, D:D + R] = C[qid[idx]]
    Qaug[sb, pos, KAUG - 1] = -BIG
    qT = np.ascontiguousarray(
        Qaug.reshape(B * QPAD, KAUG).T).astype(BF16)
    gq = np.cumsum([0] + [L * QPAD for L in GROUPS])
    qT_chunks = {}
    for c, gs in enumerate(QCHUNKS):
        qT_chunks[f"qT{c}"] = np.ascontiguousarray(
            np.concatenate([qT[:, gq[g]:gq[g + 1]] for g in gs], axis=1))

    W = mask.copy()
    W[:, 0] = 0.0  # skip [CLS]
    W2big = np.zeros((QPAD, B, NQ), np.float32)
    W2big[pos, sb, idx // LQ] = W.reshape(-1)[idx]
    w2 = np.ascontiguousarray(W2big.reshape(QPAD, B * NQ)).astype(BF16)

    in_maps = []
    for core in range(NCORES):
        dd = did[core * DSHARD:(core + 1) * DSHARD]
        dr = dreps[core * NDTOK:(core + 1) * NDTOK]
        db = (dd.reshape(-1) % B)
        docn = np.repeat(np.arange(DSHARD), LD)
        key = db * DSHARD + docn
        didx = np.argsort(key, kind="stable")
        sk = key[didx]
        dpos = np.arange(NDTOK) - np.searchsorted(sk, sk)
        # keep >= 1 all-zero slot per cell: it reproduces the reference's
        # where()-zeros so no relu pass is needed on device
        if dpos.max() > DPAD - 2:
            return None
        Daug = np.zeros((B, DSHARD, DPAD, KAUG), np.float32)
        sb2 = sk // DSHARD
        sn = sk % DSHARD
        ids_flat = dd.reshape(-1)[didx]
        Daug[sb2, sn, dpos, 0:D] = dr[didx]
        Daug[sb2, sn, dpos, D:D + R] = C[ids_flat]
        Daug[sb2, sn, dpos, KAUG - 1] = 1.0
        dT = np.ascontiguousarray(
            Daug.reshape(B * DCOLS, KAUG).T).astype(BF16)
        im = {"w2": w2}
        im.update(qT_chunks)
        gd = np.cumsum([0] + [L * DCOLS for L in GROUPS])
        for c, gs in enumerate(DCHUNKS):
            im[f"dT{c}"] = np.ascontiguousarray(
                np.concatenate([dT[:, gd[g]:gd[g + 1]] for g in gs], axis=1))
        in_maps.append(im)
    return in_maps


# ---------------------------------------------------------------------------
# Dense fallback (previous kernel): exact for any input distribution.
# ---------------------------------------------------------------------------

DR = 96                  # dense-path signature code dims
DCVAL = 4.0
DBIG = DR * DCVAL * DCVAL
DKAUG = D + DR           # 128
PA_DOCS = 4
F_DVE = 4
F_ACT = DSHARD - F_DVE
QBATCH = 4

_DENSE_CODE = None


def _dense_code():
    global _DENSE_CODE
    if _DENSE_CODE is None:
        rng = np.random.RandomState(12345)
        _DENSE_CODE = np.where(rng.rand(VOCAB, DR) < 0.5, -DCVAL,
                               DCVAL).astype(np.float32)
    return _DENSE_CODE


def _build_dense():
    from concourse import bacc, tile, mybir

    bf = mybir.dt.bfloat16
    f32 = mybir.dt.float32

    nc = bacc.Bacc("TRN2", target_bir_lowering=False, debug=False,
                   num_devices=NCORES)
    _bias_t = nc.alloc_sbuf_tensor("const-float32--1536", [128, 1],
                                   mybir.dt.float32)
    nc.gpsimd.memset(_bias_t.ap(), -float(DBIG))
    nc.const_aps.aps[(mybir.dt.float32, -float(DBIG))] = _bias_t.ap()

    qT_d = nc.declare_dram_parameter("qT", [DKAUG, NQTOK], bf, isOutput=False)
    dT_d = nc.declare_dram_parameter("dT", [DKAUG, NDTOK], bf, isOutput=False)
    w2_d = nc.declare_dram_parameter("w2", [LQ, NQ], bf, isOutput=False)
    out_d = nc.declare_dram_parameter("out", [1, NQ * DSHARD], f32,
                                      isOutput=True)

    NCHUNK = 4
    CW = NDTOK // NCHUNK

    with tile.TileContext(nc) as tc:
        with (
            tc.tile_pool(name="io", bufs=1) as io,
            tc.tile_pool(name="ebuf", bufs=2) as ebuf,
            tc.tile_pool(name="small", bufs=1) as small,
            tc.tile_pool(name="psA", bufs=2, space="PSUM") as psA,
            tc.tile_pool(name="psB", bufs=2, space="PSUM") as psB,
        ):
            scr = psB.tile([1, 16], f32, tag="psB")
            bias_ap = _bias_t.ap()
            for _ in range(48):
                nc.tensor.matmul(scr[:, 0:1], bias_ap[:, 0:1], bias_ap[:, 0:1],
                                 start=True, stop=True)

            qchunk = NQTOK // NCHUNK
            DTt = io.tile([DKAUG, NDTOK], bf, tag="dt")
            nc.sync.dma_start(DTt[:], dT_d[:])
            DTc = [DTt[:, c * CW:(c + 1) * CW] for c in range(NCHUNK)]
            QT0 = io.tile([DKAUG, qchunk], bf, tag="qt0")
            nc.scalar.dma_start(QT0[:], qT_d[:, 0:qchunk])
            QTrest = io.tile([DKAUG, NQTOK - qchunk], bf, tag="qtr")
            nc.scalar.dma_start(QTrest[:], qT_d[:, qchunk:NQTOK])
            W2 = small.tile([LQ, NQ], bf, tag="w2")
            nc.scalar.dma_start(W2[:], w2_d[:])

            Mdve = small.tile([LQ, NQ, F_DVE], f32, tag="mdve")
            Rall = small.tile([LQ, NQ, DSHARD], bf, tag="rall")
            OUTS = small.tile([1, NQ * DSHARD], f32, tag="outs")

            docs_per_chunk = CW // LD
            q_per_chunk = qchunk // LQ

            BS = [5, 5, 4, 2]
            fin_deferred = []
            deep_deferred = []

            def emit_finale(qlo, qhi):
                n = (qhi - qlo) * DSHARD
                pso_b = psA.tile([1, n], f32, tag="psA")
                for q in range(qlo, qhi):
                    nc.tensor.matmul(
                        pso_b[:, (q - qlo) * DSHARD:(q - qlo + 1) * DSHARD],
                        W2[:, q:q + 1], Rall[:, q, :],
                        start=True, stop=True)
                nc.scalar.copy(OUTS[:, qlo * DSHARD:qhi * DSHARD], pso_b[:])

            def emit_deep(T1, qlo, qhi):
                bs = qhi - qlo
                cur = T1
                width = LD // 2
                lev = 1
                while width > 2:
                    half = width // 2
                    t = ebuf.tile([LQ, bs, F_ACT, half], bf, tag=f"tr{lev}")
                    nc.vector.tensor_max(t[:], cur[:, :, :, 0:half],
                                         cur[:, :, :, half:width])
                    cur = t[:]
                    width = half
                    lev += 1
                nc.vector.tensor_max(Rall[:, qlo:qhi, F_DVE:DSHARD],
                                     cur[:, :, :, 0], cur[:, :, :, 1])

            qbase = 0
            for bi, bs in enumerate(BS):
                if fin_deferred and len(fin_deferred) > 1:
                    emit_finale(*fin_deferred.pop(0))
                E4 = ebuf.tile([LQ, bs, F_ACT, LD], bf, tag="e")
                T1 = ebuf.tile([LQ, bs, F_ACT, LD // 2], bf, tag="t1")
                for qq in range(bs):
                    q = qbase + qq
                    pA = psA.tile([LQ, PA_DOCS, LD], f32, tag="psA")
                    pB = psB.tile([LQ, DSHARD - PA_DOCS, LD], f32, tag="psB")
                    if q < q_per_chunk:
                        lhs = QT0[:, q * LQ:(q + 1) * LQ]
                    else:
                        lhs = QTrest[:, (q - q_per_chunk) * LQ:
                                     (q - q_per_chunk + 1) * LQ]
                    nc.tensor.matmul(pA[:], lhs, DTc[0],
                                     start=True, stop=True)
                    for c in range(1, NCHUNK):
                        nc.tensor.matmul(
                            pB[:, (c - 1) * docs_per_chunk:c * docs_per_chunk, :],
                            lhs, DTc[c], start=True, stop=True)
                    from concourse import mybir as _mb
                    nc.vector.reduce_max(
                        Mdve[:, q, :], pA[:, 0:F_DVE, :],
                        axis=_mb.AxisListType.X)
                    nc.scalar.activation(
                        E4[:, qq, :, :], pB[:],
                        _mb.ActivationFunctionType.Relu,
                        bias=-float(DBIG))
                    nc.vector.tensor_max(
                        T1[:, qq, :, :],
                        E4[:, qq, :, 0:LD // 2], E4[:, qq, :, LD // 2:LD])
                    if qq == 1 and deep_deferred:
                        emit_deep(*deep_deferred.pop(0))

                qlo, qhi = qbase, qbase + bs
                from concourse import mybir as _mb
                nc.scalar.activation(
                    Rall[:, qlo:qhi, 0:F_DVE], Mdve[:, qlo:qhi, :],
                    _mb.ActivationFunctionType.Relu,
                    bias=-float(DBIG))
                deep_deferred.append((T1[:], qlo, qhi))
                fin_deferred.append((qlo, qhi))
                qbase = qhi

            for args in deep_deferred:
                emit_deep(*args)
            for args in fin_deferred:
                emit_finale(*args)
            nc.sync.dma_start(out_d[:], OUTS[:])

    nc.compile()
    return nc


def _pack_dense(doc_reps, qry_reps, qry_attention_mask, doc_input_ids,
                qry_input_ids):
    C = _dense_code()
    qry_reps = np.asarray(qry_reps, dtype=np.float32)
    doc_reps = np.asarray(doc_reps, dtype=np.float32)
    mask = np.asarray(qry_attention_mask, dtype=np.float32)
    qids = np.asarray(qry_input_ids).astype(np.int64).reshape(-1)
    dids = np.asarray(doc_input_ids).astype(np.int64).reshape(-1)

    Qaug = np.concatenate(
        [qry_reps.reshape(NQTOK, D), C[qids]], axis=1).astype(BF16)
    Daug = np.concatenate(
        [doc_reps.reshape(ND * LD, D), C[dids]], axis=1).astype(BF16)
    qT = np.ascontiguousarray(Qaug.T)

    W = mask.copy()
    W[:, 0] = 0.0
    w2 = np.ascontiguousarray(W.T).astype(BF16)

    in_maps = []
    for core in range(NCORES):
        shard = Daug[core * NDTOK:(core + 1) * NDTOK]
        dT = np.ascontiguousarray(shard.T)
        in_maps.append({"qT": qT, "dT": dT, "w2": w2})
    return in_maps


# ---------------------------------------------------------------------------

_NC_BUCKET = None
_NC_DENSE = None


def _get_nc_bucketed():
    global _NC_BUCKET
    if _NC_BUCKET is None:
        _NC_BUCKET = _build_bucketed()
    return _NC_BUCKET


def _get_nc_dense():
    global _NC_DENSE
    if _NC_DENSE is None:
        _NC_DENSE = _build_dense()
    return _NC_DENSE


def _install_ntff_shim():
    """Under axon the NTFF profile hook module may be missing; install it so
    trace=True returns exec_time_ns. Harmless no-op if already present."""
    import types
    try:
        import antenv.axon_hooks  # noqa: F401
        return
    except ImportError:
        pass
    try:
        from trn_agent_boot.trn_boot import _ntff_profile_via_ctypes
        hook = _ntff_profile_via_ctypes("/opt/axon/libaxon_pjrt.so")
        mod = types.ModuleType("antenv.axon_hooks")
        mod.get_axon_ntff_profile_hook = lambda: hook
        mod.set_axon_ntff_profile_hook = lambda h: None
        sys.modules["antenv.axon_hooks"] = mod
    except Exception:
        pass


def _run(nc, in_maps, trace=False):
    from concourse.bass_utils import run_bass_kernel_spmd
    if trace:
        _install_ntff_shim()
    return run_bass_kernel_spmd(nc, in_maps, core_ids=list(range(NCORES)),
                                trace=trace)


def _kernel_impl(inputs, trace):
    in_maps = _pack_bucketed(**inputs)
    if in_maps is not None:
        res = _run(_get_nc_bucketed(), in_maps, trace=trace)
        out = np.zeros((NQ, ND), dtype=np.float32)
        for core in range(NCORES):
            out[:, core * DSHARD:(core + 1) * DSHARD] = res.results[core]["out"]
        return out, res
    in_maps = _pack_dense(**inputs)
    res = _run(_get_nc_dense(), in_maps, trace=trace)
    out = np.zeros((NQ, ND), dtype=np.float32)
    for core in range(NCORES):
        out[:, core * DSHARD:(core + 1) * DSHARD] = \
            res.results[core]["out"].reshape(NQ, DSHARD)
    return out, res


def kernel(doc_reps, qry_reps, qry_attention_mask, doc_input_ids,
           qry_input_ids):
    out, _ = _kernel_impl(dict(
        doc_reps=doc_reps, qry_reps=qry_reps,
        qry_attention_mask=qry_attention_mask,
        doc_input_ids=doc_input_ids, qry_input_ids=qry_input_ids), False)
    return out


def kernel_traced(doc_reps, qry_reps, qry_attention_mask, doc_input_ids,
                  qry_input_ids):
    """Returns (output, exec_time_ns) using the NTFF profiling path."""
    out, res = _kernel_impl(dict(
        doc_reps=doc_reps, qry_reps=qry_reps,
        qry_attention_mask=qry_attention_mask,
        doc_input_ids=doc_input_ids, qry_input_ids=qry_input_ids), True)
    return out, res.exec_time_ns
